# Initial kernel scaffold
#
"""Trainium2 Bass kernel: CACE-style GNN message passing (nn_Cace_7155415515517).

Strategy (node-parallel across 8 NeuronCores, no collectives needed):
  - Host (shard prep): sort edges by receiver, shard receivers across the
    8 cores (1280 nodes each); lay edges out in 128-edge chunks grouped
    into 128-node blocks (CPB chunks per block, padded with null edges).
    For each species z, build a z-masked one-hot matrix
    ohz_z[e, n] = (dst_e == n and z_src_e == z), sent as fp8 (exact 0/1)
    and used directly as segment-sum matmul weights. Unit edge vectors
    and the cutoff-polynomial radial weight w(r) = sqrt(2/C)*fc(r)/r are
    also computed at prep time (O(E) scalar work; all heavy tensor math
    stays on device).
  - Device, per core: Bessel radial basis sin(k*pi*r/C) via a Chebyshev
    recurrence (ACT Sin is only accurate on [-pi, pi]); angular monomials
    x^lx y^ly z^lz with sqrt(multinomial-prefactor) folded in; payload
    P[e, (r, a)] = R (x) ang (160 wide, bf16). Per node block, two PSUM
    accumulations G_z[n, ra] = sum_e ohz_z[e, n] * P[e, ra] (fp8 x bf16
    matmuls, 24 per block); then M[n, c1, ra] = sum_z G_z * W[z, c1]
    (ACT per-partition-scale mul + DVE scalar_tensor_tensor).
  - Symmetrizer uses the factorization
    A[n, r, a, c1, c2] = emb[n, c2] * M[n, r, a, c1], so
    B_0 = M[., ., a=0, .] * emb and B_l = (sum_{a in l} pref*M^2) * emb^2
    are node-local; done in 2-block slices so each slice overlaps the
    remaining blocks' matmuls, with the output DMA streamed per slice.
  - Engine balance: payload builds split DVE/GpSimd, one-hots are DMA'd
    (engines never touch them), radial/monomial prep on DVE+ACT+GpSimd.
"""
import math
import numpy as np

import concourse.bacc as bacc
import concourse.mybir as mybir
import concourse.tile as tile

AF = mybir.ActivationFunctionType
ALU = mybir.AluOpType
F32 = mybir.dt.float32
BF16 = mybir.dt.bfloat16
FP8 = mybir.dt.float8e4

N_CORES = 8
N_NODES = 10000
N_RBF = 8
NPC = 1280            # nodes per core (8*1280 = 10240, tail padded)
NBLK = 10             # 128-node blocks per core
CPB = 12              # chunks of 128 edges per block (default; grows on
                      # demand if the input degree distribution is skewed)
CUT = 5.5
SQ2C = math.sqrt(2.0 / CUT)
S2, S3, S6 = math.sqrt(2.0), math.sqrt(3.0), math.sqrt(6.0)

_CACHE = {}


def _build(cpb=CPB):
    NCH = NBLK * cpb
    CPB_ = cpb
    nc = bacc.Bacc("TRN2", target_bir_lowering=False, debug=False,
                   num_devices=N_CORES)
    r_d = nc.dram_tensor("r", [128, NCH], F32, kind="ExternalInput")
    v_d = nc.dram_tensor("v3", [128, 3 * NCH], F32, kind="ExternalInput")
    wr_d = nc.dram_tensor("wr", [128, NCH], F32, kind="ExternalInput")
    ohz_d = [nc.dram_tensor(f"ohz{z}", [128, NCH * 128], FP8,
                            kind="ExternalInput") for z in range(2)]
    e_d = nc.dram_tensor("emb", [128, 3 * NBLK], F32, kind="ExternalInput")
    w_d = nc.dram_tensor("wbc", [128, 6], F32, kind="ExternalInput")
    o_d = nc.dram_tensor("out", [128, 288 * NBLK], F32, kind="ExternalOutput")

    with tile.TileContext(nc) as tc:
        with (
            tc.tile_pool(name="mp", bufs=1) as mp,
            tc.tile_pool(name="pp", bufs=8) as pp,
            tc.tile_pool(name="ps", bufs=3, space="PSUM") as ps,
        ):
            # ---- input loads ----
            r = mp.tile([128, NCH], F32, tag="r")
            nc.sync.dma_start(r[:], r_d.ap())
            v = mp.tile([128, 3, NCH], F32, tag="v")
            nc.sync.dma_start(v[:], v_d.ap().rearrange("p (a c) -> p a c", a=3))
            wr = mp.tile([128, NCH], F32, tag="wr")
            nc.sync.dma_start(wr[:], wr_d.ap())
            wbc = mp.tile([128, 6], F32, tag="wbc")
            nc.gpsimd.dma_start(wbc[:], w_d.ap())
            emb = mp.tile([128, NBLK, 3], F32, tag="emb")
            nc.gpsimd.dma_start(emb[:], e_d.ap().rearrange("p (b c) -> p b c", b=NBLK))
            # big one-hot loads: issue from the (otherwise idle) tensor
            # engine's queue so they don't serialize behind the small inputs
            ohz = []
            for z in range(2):
                t = mp.tile([128, NCH, 128], FP8, tag=f"ohz{z}", name=f"ohz{z}")
                q = NCH // 2
                for sp in range(2):
                    nc.sync.dma_start(
                        t[:, sp * q:(sp + 1) * q],
                        ohz_d[z].ap().rearrange("p (c n) -> p c n", n=128)
                        [:, sp * q:(sp + 1) * q])
                ohz.append(t)

            one = mp.tile([128, 1], F32, tag="one")
            nc.gpsimd.memset(one[:], 1.0)
            halfpi = mp.tile([128, 1], F32, tag="halfpi")
            nc.gpsimd.memset(halfpi[:], float(np.pi / 2))

            # ---- pair products of unit vectors (unscaled, f32) ----
            ux, uy, uz = v[:, 0], v[:, 1], v[:, 2]
            t2 = mp.tile([128, 6, NCH], F32, tag="t2")
            pairs = [(0, 0), (0, 1), (0, 2), (1, 1), (1, 2), (2, 2)]
            for i, (a, b) in enumerate(pairs):
                nc.gpsimd.tensor_mul(t2[:, i], v[:, a], v[:, b])
            txx, txy, txz, tyy, tyz, tzz = (t2[:, i] for i in range(6))

            # ---- angular basis (bf16, sqrt(pref) folded) ----
            ang = mp.tile([128, 20, NCH], BF16, tag="ang")
            nc.gpsimd.memset(ang[:, 0], 1.0)
            nc.scalar.copy(ang[:, 1], ux)
            nc.scalar.copy(ang[:, 2], uy)
            nc.scalar.copy(ang[:, 3], uz)
            l2 = [(txx, 1.0), (txy, S2), (txz, S2), (tyy, 1.0), (tyz, S2), (tzz, 1.0)]
            for i, (t, s) in enumerate(l2):
                nc.scalar.mul(ang[:, 4 + i], t, s)
            l3 = [(txx, 1.0, ux), (txx, S3, uy), (txx, S3, uz),
                  (tyy, S3, ux), (txy, S6, uz), (tzz, S3, ux),
                  (tyy, 1.0, uy), (tyy, S3, uz), (tzz, S3, uy), (tzz, 1.0, uz)]
            for i, (t, s, uu) in enumerate(l3):
                nc.vector.scalar_tensor_tensor(ang[:, 10 + i], t, s, uu,
                                               op0=ALU.mult, op1=ALU.mult)

            # ---- radial basis: sin(k*pi*r/C)/r * fc(r), k=1..8 ----
            R = mp.tile([128, 8, NCH], F32, tag="R")
            nc.scalar.activation(R[:, 0], r[:], AF.Sin, scale=float(np.pi / CUT))
            cs = mp.tile([128, NCH], F32, tag="cs")
            nc.scalar.activation(cs[:], r[:], AF.Sin, scale=float(-np.pi / CUT),
                                 bias=halfpi[:])
            nc.vector.scalar_tensor_tensor(R[:, 1], cs[:], 2.0, R[:, 0],
                                           op0=ALU.mult, op1=ALU.mult)
            for k in range(2, 8):
                nc.vector.scalar_tensor_tensor(R[:, k], cs[:], 2.0, R[:, k - 1],
                                               op0=ALU.mult, op1=ALU.mult)
                nc.vector.tensor_sub(R[:, k], R[:, k], R[:, k - 2])
            nc.vector.tensor_mul(R[:], R[:],
                                 wr[:].unsqueeze(1).broadcast_to([128, 8, NCH]))

            # ---- emb^2 ----
            emb2 = mp.tile([128, NBLK, 3], F32, tag="emb2")
            nc.scalar.square(emb2[:], emb[:])

            # ---- segment-sum via z-masked one-hot matmuls ----
            # payload P[e, r, a] (160 wide) built one block per op; three
            # blocks on GpSimd to offload the DVE (bottleneck engine).
            g_all = mp.tile([128, NBLK, 2, 160], F32, tag="gall")
            for b in range(NBLK):
                c0 = b * CPB_
                pt = pp.tile([128, CPB_, 8, 20], BF16, tag="P")
                # split each block's payload build across DVE and GpSimd
                # (~7:5, matching their relative rates) so both engines
                # work every block and the matmuls start sooner
                kd = max(1, (CPB_ * 7) // 12)
                for peng, k0, k1 in ((nc.vector, 0, kd), (nc.gpsimd, kd, CPB_)):
                    nk = k1 - k0
                    if nk <= 0:
                        continue
                    peng.tensor_mul(
                        pt[:, k0:k1],
                        R[:, :, c0 + k0:c0 + k1].transpose([0, 2, 1]).unsqueeze(3)
                            .broadcast_to([128, nk, 8, 20]),
                        ang[:, :, c0 + k0:c0 + k1].transpose([0, 2, 1]).unsqueeze(2)
                            .broadcast_to([128, nk, 8, 20]))
                gs = [ps.tile([128, 160], F32, tag=f"g{z}", name=f"g{z}")
                      for z in range(2)]
                for k in range(CPB_):
                    rhs = pt[:, k].rearrange("p s a -> p (s a)")
                    for z in range(2):
                        nc.tensor.matmul(
                            gs[z][:], ohz[z][:, c0 + k], rhs,
                            start=(k == 0), stop=(k == CPB_ - 1))
                for z in range(2):
                    nc.scalar.copy(g_all[:, b, z], gs[z][:])

            # ---- post-stage: symmetrizer, done in two block-halves so
            # the first half overlaps the second half's matmuls ----
            m_all = mp.tile([128, NBLK, 3, 160], F32, tag="mall")
            Ms = mp.tile([128, NBLK, 3, 160], F32, tag="Ms")
            SM = mp.tile([128, NBLK * 3, 8, 4], F32, tag="SM")
            B = mp.tile([128, NBLK, 8, 4, 3, 3], F32, tag="B")
            M5 = m_all[:].rearrange("p b c (r a) -> p (b c) r a", r=8)
            Ms5 = Ms[:].rearrange("p b c (r a) -> p (b c) r a", r=8)
            SMv = SM[:].rearrange("p (b c) r s -> p b c r s", b=NBLK)
            PARTS_M = [2, 2, 2, 2, 2]        # M-build + square granularity
            off = 0
            for HB in PARTS_M:
                h0 = off
                off += HB
                bs = slice(h0, h0 + HB)
                # M[n, c1, r, a] = sum_z G_z * W[z, c1]
                for c1 in range(3):
                    nc.scalar.mul(m_all[:, bs, c1], g_all[:, bs, 0],
                                  wbc[:, c1:c1 + 1])
                    nc.vector.scalar_tensor_tensor(
                        m_all[:, bs, c1], g_all[:, bs, 1],
                        wbc[:, 3 + c1:4 + c1],
                        m_all[:, bs, c1], op0=ALU.mult, op1=ALU.add)
                nc.scalar.square(Ms[:, bs], m_all[:, bs])
            PARTS = [2, 2, 2, 2, 2]   # symmetrizer + output granularity
            off = 0
            for HB in PARTS:
                h0 = off
                off += HB
                bs = slice(h0, h0 + HB)
                fs = slice(h0 * 3, (h0 + HB) * 3)   # fused (b c1) rows
                nc.gpsimd.tensor_copy(SM[:, fs, :, 0], M5[:, fs, :, 0])
                nc.vector.tensor_reduce(SM[:, fs, :, 1], Ms5[:, fs, :, 1:4],
                                        axis=mybir.AxisListType.X, op=ALU.add)
                nc.vector.tensor_reduce(SM[:, fs, :, 2], Ms5[:, fs, :, 4:10],
                                        axis=mybir.AxisListType.X, op=ALU.add)
                nc.vector.tensor_reduce(SM[:, fs, :, 3], Ms5[:, fs, :, 10:20],
                                        axis=mybir.AxisListType.X, op=ALU.add)
                # B[p, b, r, l, c1, c2]
                for l in range(4):
                    efac = emb if l == 0 else emb2
                    for c1 in range(3):
                        (nc.vector if l == 0 else nc.gpsimd).tensor_mul(
                            B[:, bs, :, l, c1],
                            SMv[:, bs, c1, :, l].unsqueeze(3).broadcast_to(
                                [128, HB, 8, 3]),
                            efac[:, bs].unsqueeze(2).broadcast_to([128, HB, 8, 3]))
                nc.sync.dma_start(
                    o_d.ap()[:, h0 * 288:(h0 + HB) * 288],
                    B[:, bs].rearrange("p b r l c d -> p (b r l c d)"))

    nc.compile()
    return nc


def _host_prep(inputs, cpb=CPB):
    NCH = NBLK * cpb
    import ml_dtypes
    bf16 = ml_dtypes.bfloat16
    fp8 = ml_dtypes.float8_e4m3

    an = np.asarray(inputs["atomic_numbers"]).astype(np.int64)
    ei = np.asarray(inputs["edge_index"]).astype(np.int64)
    el = np.asarray(inputs["edge_lengths"]).astype(np.float32)
    ev = np.asarray(inputs["edge_vectors"]).astype(np.float32)
    W = np.asarray(inputs["W_embed"]).astype(np.float32)

    emb = W[an]                                     # [N, 3]
    src, dst = ei[0], ei[1]
    z_src = an[src]
    order = np.argsort(dst, kind="stable")
    dst_s, el_s, ev_s, zs_s = dst[order], el[order], ev[order], z_src[order]
    wbc = np.ascontiguousarray(
        np.broadcast_to(W.reshape(-1), (128, 6))).astype(np.float32)

    in_maps = []
    for c in range(N_CORES):
        lo, hi = c * NPC, (c + 1) * NPC
        lo_i = np.searchsorted(dst_s, lo, "left")
        hi_i = np.searchsorted(dst_s, min(hi, N_NODES), "left")
        d_l = dst_s[lo_i:hi_i] - lo
        e_l, v_l, z_l = el_s[lo_i:hi_i], ev_s[lo_i:hi_i], zs_s[lo_i:hi_i]

        S = NCH * 128
        r_pad = np.ones(S, np.float32)
        w_pad = np.zeros(S, np.float32)
        v_pad = np.zeros((S, 3), np.float32)
        v_pad[:, 0] = 1.0
        # slot index for each real edge (block-padded layout)
        blk = (d_l // 128).astype(np.int64)
        bounds = np.searchsorted(blk, np.arange(NBLK + 1), "left")
        slot = np.empty(len(d_l), np.int64)
        for b in range(NBLK):
            s0, s1 = int(bounds[b]), int(bounds[b + 1])
            cnt = s1 - s0
            assert cnt <= cpb * 128
            slot[s0:s1] = b * cpb * 128 + np.arange(cnt)
        r_pad[slot] = e_l
        # unit vectors and cutoff-polynomial radial weight, computed at
        # shard-prep time (exact f32, O(E) scalar work)
        nv = np.sqrt((v_l * v_l).sum(1))
        nv[nv == 0] = 1.0
        v_pad[slot] = v_l / nv[:, None]
        uu = e_l / np.float32(CUT)
        fcut = (1.0 - 28.0 * uu**6 + 48.0 * uu**7 - 21.0 * uu**8) * (uu < 1.0)
        w_pad[slot] = np.float32(SQ2C) * fcut / e_l

        # device layout [128, NCH]: edge i of chunk k at [i, k]
        def lay(x):
            return np.ascontiguousarray(x.reshape(NCH, 128).T)

        v_lay = np.stack([lay(v_pad[:, 0]), lay(v_pad[:, 1]), lay(v_pad[:, 2])], 1)

        # z-masked one-hots: ohz[z][e, chunk, n] = (z_e == z) at n = dst rel
        e_idx = slot % 128
        c_idx = slot // 128
        n_idx = d_l % 128
        ohz_list = []
        for z in range(2):
            arr = np.zeros((128, NCH, 128), fp8)
            m = z_l == z
            arr[e_idx[m], c_idx[m], n_idx[m]] = 1.0
            ohz_list.append(arr.reshape(128, NCH * 128))

        emb_core = np.zeros((NPC, 3), np.float32)
        n_real = max(0, min(hi, N_NODES) - lo)
        emb_core[:n_real] = emb[lo:lo + n_real]
        emb_lay = np.ascontiguousarray(
            emb_core.reshape(NBLK, 128, 3).transpose(1, 0, 2).reshape(128, NBLK * 3))

        in_maps.append(dict(
            r=lay(r_pad),
            wr=lay(w_pad),
            v3=np.ascontiguousarray(v_lay.reshape(128, 3 * NCH)),
            ohz0=ohz_list[0], ohz1=ohz_list[1],
            emb=emb_lay, wbc=wbc,
        ))
    return in_maps


def _make_runner(nc):
    """Cached-jit version of run_bass_kernel_spmd's axon execution path
    (bass2jax.run_bass_via_pjrt): one jitted shard_map over 8 NeuronCores,
    reused across kernel() calls instead of re-tracing every call."""
    import jax
    from concourse import bass2jax
    from jax.experimental.shard_map import shard_map
    from jax.sharding import Mesh, PartitionSpec

    bass2jax.install_neuronx_cc_hook()
    partition_name = (nc.partition_id_tensor.name
                      if nc.partition_id_tensor else None)
    in_names, out_names, out_avals = [], [], []
    for alloc in nc.m.functions[0].allocations:
        if not isinstance(alloc, mybir.MemoryLocationSet):
            continue
        name = alloc.memorylocations[0].name
        if alloc.kind == "ExternalInput":
            if name != partition_name:
                in_names.append(name)
        elif alloc.kind == "ExternalOutput":
            out_names.append(name)
            out_avals.append(jax.core.ShapedArray(
                tuple(alloc.tensor_shape), mybir.dt.np(alloc.dtype)))
    n_params, n_outs = len(in_names), len(out_names)
    all_in_names = list(in_names) + list(out_names)
    if partition_name is not None:
        all_in_names.append(partition_name)

    def _body(*args):
        operands = list(args)
        if partition_name is not None:
            operands.append(bass2jax.partition_id_tensor())
        outs = bass2jax._bass_exec_p.bind(
            *operands,
            out_avals=tuple(out_avals),
            in_names=tuple(all_in_names),
            out_names=tuple(out_names),
            lowering_input_output_aliases=(),
            sim_require_finite=True,
            sim_require_nnan=True,
            nc=nc)
        return tuple(outs)

    devices = jax.devices()[:N_CORES]
    mesh = Mesh(np.asarray(devices), ("core",))
    in_specs = (PartitionSpec("core"),) * (n_params + n_outs)
    out_specs = (PartitionSpec("core"),) * n_outs
    sharded = jax.jit(
        shard_map(_body, mesh=mesh, in_specs=in_specs, out_specs=out_specs,
                  check_rep=False),
        keep_unused=True)
    # zero output-seed buffers, resident on device, reused every call
    # (no donation, so they are never consumed)
    from jax.sharding import NamedSharding
    zero_outs = [
        jax.device_put(
            np.zeros((N_CORES * a.shape[0], *a.shape[1:]), a.dtype),
            NamedSharding(mesh, PartitionSpec("core")))
        for a in out_avals]
    return sharded, in_names, out_names, out_avals, zero_outs


def _max_block_edges(inputs):
    dst = np.asarray(inputs["edge_index"]).astype(np.int64)[1]
    return int(np.bincount(dst // 128, minlength=80).max())


def _run(in_maps, cpb):
    key = ("runner", cpb)
    if key not in _CACHE:
        nc = _build(cpb)
        _CACHE[("nc", cpb)] = nc
        _CACHE[key] = _make_runner(nc)
    sharded, in_names, out_names, out_avals, zero_outs = _CACHE[key]
    concat_in = [np.concatenate([m[nm] for m in in_maps], 0) for nm in in_names]
    outs = sharded(*concat_in, *zero_outs)
    return np.asarray(outs[0])          # [8*128, 2880]


def kernel(**inputs):
    cpb = max(CPB, -(-_max_block_edges(inputs) // 128))
    if cpb > 24:
        raise RuntimeError(f"receiver-degree skew too large: cpb={cpb}")
    in_maps = _host_prep(inputs, cpb)
    raw = _run(in_maps, cpb)
    parts = []
    for c in range(N_CORES):
        o = raw[c * 128:(c + 1) * 128]               # [128, 2880]
        parts.append(o.reshape(128, NBLK, 288).transpose(1, 0, 2).reshape(NPC, 288))
    full = np.concatenate(parts, 0)[:N_NODES]
    return np.ascontiguousarray(full.reshape(N_NODES, N_RBF, 4, 9)).astype(np.float32)



# revision 1
# speedup vs baseline: 1.0571x; 1.0571x over previous
"""Trainium2 Bass kernel: CACE-style GNN message passing (nn_Cace_7155415515517).

Strategy (node-parallel across 8 NeuronCores, no collectives needed):
  - Host (shard prep): sort edges by receiver, shard receivers across the
    8 cores (1280 nodes each); lay edges out in 128-edge chunks grouped
    into 128-node blocks (CPB chunks per block, padded with null edges).
    For each species z, build a z-masked one-hot matrix
    ohz_z[e, n] = (dst_e == n and z_src_e == z), sent as fp8 (exact 0/1)
    and used directly as segment-sum matmul weights. Unit edge vectors
    and the cutoff-polynomial radial weight w(r) = sqrt(2/C)*fc(r)/r are
    also computed at prep time (O(E) scalar work; all heavy tensor math
    stays on device).
  - Device, per core: Bessel radial basis sin(k*pi*r/C) via a Chebyshev
    recurrence (ACT Sin is only accurate on [-pi, pi]); angular monomials
    x^lx y^ly z^lz with sqrt(multinomial-prefactor) folded in; payload
    P[e, (r, a)] = R (x) ang (160 wide, bf16). Per node block, two PSUM
    accumulations G_z[n, ra] = sum_e ohz_z[e, n] * P[e, ra] (fp8 x bf16
    matmuls, 24 per block); then M[n, c1, ra] = sum_z G_z * W[z, c1]
    (ACT per-partition-scale mul + DVE scalar_tensor_tensor).
  - Symmetrizer uses the factorization
    A[n, r, a, c1, c2] = emb[n, c2] * M[n, r, a, c1], so
    B_0 = M[., ., a=0, .] * emb and B_l = (sum_{a in l} pref*M^2) * emb^2
    are node-local; done in 2-block slices so each slice overlaps the
    remaining blocks' matmuls, with the output DMA streamed per slice.
  - Engine balance: payload builds split DVE/GpSimd, one-hots are DMA'd
    (engines never touch them), radial/monomial prep on DVE+ACT+GpSimd.
"""
import math
import numpy as np

import concourse.bacc as bacc
import concourse.mybir as mybir
import concourse.tile as tile

AF = mybir.ActivationFunctionType
ALU = mybir.AluOpType
F32 = mybir.dt.float32
BF16 = mybir.dt.bfloat16
FP8 = mybir.dt.float8e4

N_CORES = 8
N_NODES = 10000
N_RBF = 8
NPC = 1280            # nodes per core (8*1280 = 10240, tail padded)
NBLK = 10             # 128-node blocks per core
CPB = 12              # chunks of 128 edges per block (default; grows on
                      # demand if the input degree distribution is skewed)
CUT = 5.5
SQ2C = math.sqrt(2.0 / CUT)
S2, S3, S6 = math.sqrt(2.0), math.sqrt(3.0), math.sqrt(6.0)

_CACHE = {}


def _build(cpb=CPB):
    NCH = NBLK * cpb
    CPB_ = cpb
    nc = bacc.Bacc("TRN2", target_bir_lowering=False, debug=False,
                   num_devices=N_CORES)
    r_d = nc.dram_tensor("r", [128, NCH], F32, kind="ExternalInput")
    v_d = nc.dram_tensor("v3", [128, 3 * NCH], F32, kind="ExternalInput")
    wr_d = nc.dram_tensor("wr", [128, NCH], F32, kind="ExternalInput")
    ohz_d = [nc.dram_tensor(f"ohz{z}", [128, NCH * 128], FP8,
                            kind="ExternalInput") for z in range(2)]
    e_d = nc.dram_tensor("emb", [128, 3 * NBLK], F32, kind="ExternalInput")
    w_d = nc.dram_tensor("wbc", [128, 6], F32, kind="ExternalInput")
    o_d = nc.dram_tensor("out", [128, 288 * NBLK], F32, kind="ExternalOutput")

    with tile.TileContext(nc) as tc:
        with (
            tc.tile_pool(name="mp", bufs=1) as mp,
            tc.tile_pool(name="pp", bufs=8) as pp,
            tc.tile_pool(name="ps", bufs=3, space="PSUM") as ps,
        ):
            # ---- input loads ----
            r = mp.tile([128, NCH], F32, tag="r")
            nc.sync.dma_start(r[:], r_d.ap())
            v = mp.tile([128, 3, NCH], F32, tag="v")
            nc.sync.dma_start(v[:], v_d.ap().rearrange("p (a c) -> p a c", a=3))
            wr = mp.tile([128, NCH], F32, tag="wr")
            nc.sync.dma_start(wr[:], wr_d.ap())
            wbc = mp.tile([128, 6], F32, tag="wbc")
            nc.gpsimd.dma_start(wbc[:], w_d.ap())
            emb = mp.tile([128, NBLK, 3], F32, tag="emb")
            nc.gpsimd.dma_start(emb[:], e_d.ap().rearrange("p (b c) -> p b c", b=NBLK))
            # big one-hot loads: issue from the (otherwise idle) tensor
            # engine's queue so they don't serialize behind the small inputs
            ohz = []
            for z in range(2):
                t = mp.tile([128, NCH, 128], FP8, tag=f"ohz{z}", name=f"ohz{z}")
                q = NCH // 2
                for sp in range(2):
                    nc.sync.dma_start(
                        t[:, sp * q:(sp + 1) * q],
                        ohz_d[z].ap().rearrange("p (c n) -> p c n", n=128)
                        [:, sp * q:(sp + 1) * q])
                ohz.append(t)

            one = mp.tile([128, 1], F32, tag="one")
            nc.gpsimd.memset(one[:], 1.0)
            halfpi = mp.tile([128, 1], F32, tag="halfpi")
            nc.gpsimd.memset(halfpi[:], float(np.pi / 2))

            # ---- pair products of unit vectors (unscaled, f32) ----
            ux, uy, uz = v[:, 0], v[:, 1], v[:, 2]
            t2 = mp.tile([128, 6, NCH], F32, tag="t2")
            pairs = [(0, 0), (0, 1), (0, 2), (1, 1), (1, 2), (2, 2)]
            for i, (a, b) in enumerate(pairs):
                nc.gpsimd.tensor_mul(t2[:, i], v[:, a], v[:, b])
            txx, txy, txz, tyy, tyz, tzz = (t2[:, i] for i in range(6))

            # ---- angular basis (bf16, sqrt(pref) folded) ----
            ang = mp.tile([128, 20, NCH], BF16, tag="ang")
            nc.gpsimd.memset(ang[:, 0], 1.0)
            nc.scalar.copy(ang[:, 1], ux)
            nc.scalar.copy(ang[:, 2], uy)
            nc.scalar.copy(ang[:, 3], uz)
            l2 = [(txx, 1.0), (txy, S2), (txz, S2), (tyy, 1.0), (tyz, S2), (tzz, 1.0)]
            for i, (t, s) in enumerate(l2):
                nc.scalar.mul(ang[:, 4 + i], t, s)
            l3 = [(txx, 1.0, ux), (txx, S3, uy), (txx, S3, uz),
                  (tyy, S3, ux), (txy, S6, uz), (tzz, S3, ux),
                  (tyy, 1.0, uy), (tyy, S3, uz), (tzz, S3, uy), (tzz, 1.0, uz)]
            for i, (t, s, uu) in enumerate(l3):
                nc.vector.scalar_tensor_tensor(ang[:, 10 + i], t, s, uu,
                                               op0=ALU.mult, op1=ALU.mult)

            # ---- radial basis: sin(k*pi*r/C)/r * fc(r), k=1..8 ----
            R = mp.tile([128, 8, NCH], F32, tag="R")
            nc.scalar.activation(R[:, 0], r[:], AF.Sin, scale=float(np.pi / CUT))
            cs = mp.tile([128, NCH], F32, tag="cs")
            nc.scalar.activation(cs[:], r[:], AF.Sin, scale=float(-np.pi / CUT),
                                 bias=halfpi[:])
            nc.vector.scalar_tensor_tensor(R[:, 1], cs[:], 2.0, R[:, 0],
                                           op0=ALU.mult, op1=ALU.mult)
            for k in range(2, 8):
                nc.vector.scalar_tensor_tensor(R[:, k], cs[:], 2.0, R[:, k - 1],
                                               op0=ALU.mult, op1=ALU.mult)
                nc.vector.tensor_sub(R[:, k], R[:, k], R[:, k - 2])
            nc.vector.tensor_mul(R[:], R[:],
                                 wr[:].unsqueeze(1).broadcast_to([128, 8, NCH]))

            # ---- emb^2 ----
            emb2 = mp.tile([128, NBLK, 3], F32, tag="emb2")
            nc.scalar.square(emb2[:], emb[:])

            # ---- segment-sum via z-masked one-hot matmuls ----
            # payload P[e, r, a] (160 wide) built one block per op; three
            # blocks on GpSimd to offload the DVE (bottleneck engine).
            g_all = mp.tile([128, NBLK, 2, 160], F32, tag="gall")
            for b in range(NBLK):
                c0 = b * CPB_
                pt = pp.tile([128, CPB_, 8, 20], BF16, tag="P")
                # split each block's payload build across DVE and GpSimd
                # (~7:5, matching their relative rates) so both engines
                # work every block and the matmuls start sooner
                kd = max(1, (CPB_ * 7) // 12)
                for peng, k0, k1 in ((nc.vector, 0, kd), (nc.gpsimd, kd, CPB_)):
                    nk = k1 - k0
                    if nk <= 0:
                        continue
                    peng.tensor_mul(
                        pt[:, k0:k1],
                        R[:, :, c0 + k0:c0 + k1].transpose([0, 2, 1]).unsqueeze(3)
                            .broadcast_to([128, nk, 8, 20]),
                        ang[:, :, c0 + k0:c0 + k1].transpose([0, 2, 1]).unsqueeze(2)
                            .broadcast_to([128, nk, 8, 20]))
                gs = [ps.tile([128, 160], F32, tag=f"g{z}", name=f"g{z}")
                      for z in range(2)]
                for k in range(CPB_):
                    rhs = pt[:, k].rearrange("p s a -> p (s a)")
                    for z in range(2):
                        nc.tensor.matmul(
                            gs[z][:], ohz[z][:, c0 + k], rhs,
                            start=(k == 0), stop=(k == CPB_ - 1))
                for z in range(2):
                    nc.scalar.copy(g_all[:, b, z], gs[z][:])

            # ---- post-stage: symmetrizer, done in two block-halves so
            # the first half overlaps the second half's matmuls ----
            m_all = mp.tile([128, NBLK, 3, 160], F32, tag="mall")
            Ms = mp.tile([128, NBLK, 3, 160], F32, tag="Ms")
            SM = mp.tile([128, NBLK * 3, 8, 4], F32, tag="SM")
            B = mp.tile([128, NBLK, 8, 4, 3, 3], F32, tag="B")
            M5 = m_all[:].rearrange("p b c (r a) -> p (b c) r a", r=8)
            Ms5 = Ms[:].rearrange("p b c (r a) -> p (b c) r a", r=8)
            SMv = SM[:].rearrange("p (b c) r s -> p b c r s", b=NBLK)
            PARTS_M = [2, 2, 2, 2, 2]        # M-build + square granularity
            off = 0
            for HB in PARTS_M:
                h0 = off
                off += HB
                bs = slice(h0, h0 + HB)
                # M[n, c1, r, a] = sum_z G_z * W[z, c1]
                for c1 in range(3):
                    nc.scalar.mul(m_all[:, bs, c1], g_all[:, bs, 0],
                                  wbc[:, c1:c1 + 1])
                    nc.vector.scalar_tensor_tensor(
                        m_all[:, bs, c1], g_all[:, bs, 1],
                        wbc[:, 3 + c1:4 + c1],
                        m_all[:, bs, c1], op0=ALU.mult, op1=ALU.add)
                nc.scalar.square(Ms[:, bs], m_all[:, bs])
            PARTS = [2, 2, 2, 2, 2]   # symmetrizer + output granularity
            off = 0
            for HB in PARTS:
                h0 = off
                off += HB
                bs = slice(h0, h0 + HB)
                fs = slice(h0 * 3, (h0 + HB) * 3)   # fused (b c1) rows
                nc.gpsimd.tensor_copy(SM[:, fs, :, 0], M5[:, fs, :, 0])
                nc.vector.tensor_reduce(SM[:, fs, :, 1], Ms5[:, fs, :, 1:4],
                                        axis=mybir.AxisListType.X, op=ALU.add)
                nc.vector.tensor_reduce(SM[:, fs, :, 2], Ms5[:, fs, :, 4:10],
                                        axis=mybir.AxisListType.X, op=ALU.add)
                nc.vector.tensor_reduce(SM[:, fs, :, 3], Ms5[:, fs, :, 10:20],
                                        axis=mybir.AxisListType.X, op=ALU.add)
                # B[p, b, r, l, c1, c2]
                for l in range(4):
                    efac = emb if l == 0 else emb2
                    for c1 in range(3):
                        (nc.vector if l == 0 else nc.gpsimd).tensor_mul(
                            B[:, bs, :, l, c1],
                            SMv[:, bs, c1, :, l].unsqueeze(3).broadcast_to(
                                [128, HB, 8, 3]),
                            efac[:, bs].unsqueeze(2).broadcast_to([128, HB, 8, 3]))
                nc.sync.dma_start(
                    o_d.ap()[:, h0 * 288:(h0 + HB) * 288],
                    B[:, bs].rearrange("p b r l c d -> p (b r l c d)"))

    nc.compile()
    return nc


def _host_prep(inputs, cpb=CPB):
    NCH = NBLK * cpb
    import ml_dtypes
    bf16 = ml_dtypes.bfloat16
    fp8 = ml_dtypes.float8_e4m3

    an = np.asarray(inputs["atomic_numbers"]).astype(np.int64)
    ei = np.asarray(inputs["edge_index"]).astype(np.int64)
    el = np.asarray(inputs["edge_lengths"]).astype(np.float32)
    ev = np.asarray(inputs["edge_vectors"]).astype(np.float32)
    W = np.asarray(inputs["W_embed"]).astype(np.float32)

    emb = W[an]                                     # [N, 3]
    src, dst = ei[0], ei[1]
    z_src = an[src]
    order = np.argsort(dst, kind="stable")
    dst_s, el_s, ev_s, zs_s = dst[order], el[order], ev[order], z_src[order]
    wbc = np.ascontiguousarray(
        np.broadcast_to(W.reshape(-1), (128, 6))).astype(np.float32)

    in_maps = []
    for c in range(N_CORES):
        lo, hi = c * NPC, (c + 1) * NPC
        lo_i = np.searchsorted(dst_s, lo, "left")
        hi_i = np.searchsorted(dst_s, min(hi, N_NODES), "left")
        d_l = dst_s[lo_i:hi_i] - lo
        e_l, v_l, z_l = el_s[lo_i:hi_i], ev_s[lo_i:hi_i], zs_s[lo_i:hi_i]

        S = NCH * 128
        r_pad = np.ones(S, np.float32)
        w_pad = np.zeros(S, np.float32)
        v_pad = np.zeros((S, 3), np.float32)
        v_pad[:, 0] = 1.0
        # slot index for each real edge (block-padded layout)
        blk = (d_l // 128).astype(np.int64)
        bounds = np.searchsorted(blk, np.arange(NBLK + 1), "left")
        slot = np.empty(len(d_l), np.int64)
        for b in range(NBLK):
            s0, s1 = int(bounds[b]), int(bounds[b + 1])
            cnt = s1 - s0
            assert cnt <= cpb * 128
            slot[s0:s1] = b * cpb * 128 + np.arange(cnt)
        r_pad[slot] = e_l
        # unit vectors and cutoff-polynomial radial weight, computed at
        # shard-prep time (exact f32, O(E) scalar work)
        nv = np.sqrt((v_l * v_l).sum(1))
        nv[nv == 0] = 1.0
        v_pad[slot] = v_l / nv[:, None]
        uu = e_l / np.float32(CUT)
        fcut = (1.0 - 28.0 * uu**6 + 48.0 * uu**7 - 21.0 * uu**8) * (uu < 1.0)
        w_pad[slot] = np.float32(SQ2C) * fcut / e_l

        # device layout [128, NCH]: edge i of chunk k at [i, k]
        def lay(x):
            return np.ascontiguousarray(x.reshape(NCH, 128).T)

        v_lay = np.stack([lay(v_pad[:, 0]), lay(v_pad[:, 1]), lay(v_pad[:, 2])], 1)

        # z-masked one-hots: ohz[z][e, chunk, n] = (z_e == z) at n = dst rel
        e_idx = slot % 128
        c_idx = slot // 128
        n_idx = d_l % 128
        ohz_list = []
        for z in range(2):
            arr = np.zeros((128, NCH, 128), fp8)
            m = z_l == z
            arr[e_idx[m], c_idx[m], n_idx[m]] = 1.0
            ohz_list.append(arr.reshape(128, NCH * 128))

        emb_core = np.zeros((NPC, 3), np.float32)
        n_real = max(0, min(hi, N_NODES) - lo)
        emb_core[:n_real] = emb[lo:lo + n_real]
        emb_lay = np.ascontiguousarray(
            emb_core.reshape(NBLK, 128, 3).transpose(1, 0, 2).reshape(128, NBLK * 3))

        in_maps.append(dict(
            r=lay(r_pad),
            wr=lay(w_pad),
            v3=np.ascontiguousarray(v_lay.reshape(128, 3 * NCH)),
            ohz0=ohz_list[0], ohz1=ohz_list[1],
            emb=emb_lay, wbc=wbc,
        ))
    return in_maps


def _make_runner(nc):
    """Cached-jit version of run_bass_kernel_spmd's axon execution path
    (bass2jax.run_bass_via_pjrt): one jitted shard_map over 8 NeuronCores,
    reused across kernel() calls instead of re-tracing every call."""
    import jax
    from concourse import bass2jax
    from jax.experimental.shard_map import shard_map
    from jax.sharding import Mesh, PartitionSpec

    bass2jax.install_neuronx_cc_hook()
    partition_name = (nc.partition_id_tensor.name
                      if nc.partition_id_tensor else None)
    in_names, out_names, out_avals = [], [], []
    for alloc in nc.m.functions[0].allocations:
        if not isinstance(alloc, mybir.MemoryLocationSet):
            continue
        name = alloc.memorylocations[0].name
        if alloc.kind == "ExternalInput":
            if name != partition_name:
                in_names.append(name)
        elif alloc.kind == "ExternalOutput":
            out_names.append(name)
            out_avals.append(jax.core.ShapedArray(
                tuple(alloc.tensor_shape), mybir.dt.np(alloc.dtype)))
    n_params, n_outs = len(in_names), len(out_names)
    all_in_names = list(in_names) + list(out_names)
    if partition_name is not None:
        all_in_names.append(partition_name)

    def _body(*args):
        operands = list(args)
        if partition_name is not None:
            operands.append(bass2jax.partition_id_tensor())
        outs = bass2jax._bass_exec_p.bind(
            *operands,
            out_avals=tuple(out_avals),
            in_names=tuple(all_in_names),
            out_names=tuple(out_names),
            lowering_input_output_aliases=(),
            sim_require_finite=True,
            sim_require_nnan=True,
            nc=nc)
        return tuple(outs)

    devices = jax.devices()[:N_CORES]
    mesh = Mesh(np.asarray(devices), ("core",))
    in_specs = (PartitionSpec("core"),) * (n_params + n_outs)
    out_specs = (PartitionSpec("core"),) * n_outs
    sharded = jax.jit(
        shard_map(_body, mesh=mesh, in_specs=in_specs, out_specs=out_specs,
                  check_rep=False),
        keep_unused=True)
    # zero output-seed buffers, resident on device, reused every call
    # (no donation, so they are never consumed)
    from jax.sharding import NamedSharding
    zero_outs = [
        jax.device_put(
            np.zeros((N_CORES * a.shape[0], *a.shape[1:]), a.dtype),
            NamedSharding(mesh, PartitionSpec("core")))
        for a in out_avals]
    return sharded, in_names, out_names, out_avals, zero_outs


def _max_block_edges(inputs):
    dst = np.asarray(inputs["edge_index"]).astype(np.int64)[1]
    return int(np.bincount(dst // 128, minlength=80).max())


def _run(in_maps, cpb):
    key = ("runner", cpb)
    if key not in _CACHE:
        nc = _build(cpb)
        _CACHE[("nc", cpb)] = nc
        _CACHE[key] = _make_runner(nc)
    sharded, in_names, out_names, out_avals, zero_outs = _CACHE[key]
    concat_in = [np.concatenate([m[nm] for m in in_maps], 0) for nm in in_names]
    outs = sharded(*concat_in, *zero_outs)
    return np.asarray(outs[0])          # [8*128, 2880]


def kernel(**inputs):
    cpb = max(CPB, -(-_max_block_edges(inputs) // 128))
    if cpb > 24:
        raise RuntimeError(f"receiver-degree skew too large: cpb={cpb}")
    in_maps = _host_prep(inputs, cpb)
    raw = _run(in_maps, cpb)
    parts = []
    for c in range(N_CORES):
        o = raw[c * 128:(c + 1) * 128]               # [128, 2880]
        parts.append(o.reshape(128, NBLK, 288).transpose(1, 0, 2).reshape(NPC, 288))
    full = np.concatenate(parts, 0)[:N_NODES]
    return np.ascontiguousarray(full.reshape(N_NODES, N_RBF, 4, 9)).astype(np.float32)

